# revision 1
# baseline (speedup 1.0000x reference)
"""Causal self-attention with relative position (skew trick), 8-way
head-sharded across Trainium2 NeuronCores.

Shapes (hardcoded): x [4, 2048, 1024], W_attn [1024, 3072], b_attn [3072],
Er [2048, 64], W_proj [1024, 1024], b_proj [1024].  16 heads of 64; each of
the 8 cores handles 2 heads (A, B) for all 4 batches and emits a partial
(pre-reduce) projection output; the host sums the 8 partials (the
tensor-parallel unshard).  b_proj is fed only to core 0 so the sum adds it
exactly once.

Per-core pipeline (per batch):
  1. qkv^T on PE: psum = W_c^T x^T with bf16 x/W inputs (halves input DMA);
     the qkv bias is applied in the psum->SBUF copies via DVE tensor_scalar
     with per-partition bias columns (no bias-init matmuls); q is pre-scaled
     by 1/sqrt(64); v is PE-transposed into natural [token, hs] layout with
     an appended ones column (denominator trick).
  2. U = scale*(q . Er^T) per head -> DRAM f16 (triangular region only),
     rows of both heads interleaved, row pitch UP=L+512 with pad columns
     preset to -60000 (== causal mask after exp).
  3. S^T tiles [key,query] via packed K=64 matmuls (head A rows 0-63, head
     B rows 64-127 run concurrently in separate PE row-groups); the skewed
     rel-pos tile is read straight from U with a strided *transposing* DMA
     (xbar, f16) and accumulated into the S psum by an identity matmul;
     exp on ACT reads the psum directly.  No max-subtraction: |logits|<~6.
     NOTE: srel transposes and U-pad writes MUST stay on their queues
     (sync / gpsimd); issuing them from nc.scalar / nc.sync corrupts
     results on hardware even though CoreSim passes.
  4. PV with V' stationary: psum[65,512] rows 0-63 = unnormalized y^T,
     row 64 = sum(exp).  Normalize into yn via reciprocal + K=1 broadcast
     matmul.
  5. Partial projection: K=128 matmuls per token block; psum copied to an
     f16 tile (halves output DMA); the host sums the 8 f16 partials in f32
     and adds b_proj there.

Attention matmuls use float32r (1s+8e+11m, 4x faster than fp32 on the PE);
qkv uses bf16 inputs.  build_program(reps=R) wraps the whole body in a
tc.For_i hardware loop -- used by test.py to measure per-iteration device
time as a slope, cancelling the ~50-100ms axon dispatch overhead.
"""

import numpy as np
from contextlib import ExitStack

import concourse.bass as bass
import concourse.tile as tile
from concourse import mybir
from concourse import bass_utils
from concourse.masks import make_identity
from concourse import library_config

B, L, D = 4, 2048, 1024
NH, HS = 16, 64
NCORES = 8
HPC = 2                 # heads per core
CW = HPC * HS           # 128 head-cols per core
SCALE = 1.0 / 8.0       # 1/sqrt(HS)
F32 = mybir.dt.float32
F32R = mybir.dt.float32r
F16 = mybir.dt.float16
BF16 = mybir.dt.bfloat16
TOKS = B * L
UP = L + 512            # U row pitch; cols [L, UP) = causal-mask pad
NT = L // 128           # token blocks per batch


# walrus in this toolchain rejects instructions carrying >1 sync-wait;
# move excess waits onto preceding same-engine NOPs.
def _split_excess_waits(nc, max_waits=1):
    for f in nc.m.functions:
        for blk in f.blocks:
            new_insts = []
            for inst in blk.instructions:
                si = getattr(inst, "sync_info", None)
                if si is not None and si.on_wait and len(si.on_wait) > max_waits:
                    waits = list(si.on_wait)
                    chunks = [waits[i:i + max_waits]
                              for i in range(0, len(waits), max_waits)]
                    for j, ch in enumerate(chunks[:-1]):
                        new_insts.append(mybir.InstNoOp(
                            name=f"{inst.name}-waitsplit{j}",
                            engine=inst.engine,
                            sync_info=mybir.SyncInfo(on_wait=ch, on_update=[]),
                            bass_nofuse=True,
                        ))
                    si.on_wait = chunks[-1]
                new_insts.append(inst)
            blk.instructions[:] = new_insts


def jb_min(ib):
    # U row-block ib (128 rows at i0=128*ib) needs Er-index columns
    # j >= 2047 - (i0+127); 512-wide column blocks from jb_min(ib) to 3.
    return max(0, (1920 - 128 * ib) // 512)


def build_program(phases=("qkv", "u", "attn", "proj"), reps=1):
    nc = bass.Bass("TRN2", target_bir_lowering=False, debug=False,
                   num_devices=NCORES)
    xT = nc.declare_dram_parameter("xT", [D, TOKS], BF16, isOutput=False)
    wqkv = nc.declare_dram_parameter("wqkv", [D, 3 * CW], BF16, isOutput=False)
    bqkv = nc.declare_dram_parameter("bqkv", [3 * CW], F32, isOutput=False)
    ertd = nc.declare_dram_parameter("ertd", [2 * HS, L], F32R, isOutput=False)
    wp = nc.declare_dram_parameter("wp", [CW, D], F32R, isOutput=False)
    part = nc.declare_dram_parameter("part", [TOKS, D], F16, isOutput=True)
    # U scratch: [slot(batch%2), i, head, col]; pitch UP, pad cols hold the
    # causal mask.  Interleaving heads lets one DMA write both heads' tiles.
    u_scr = nc.dram_tensor("u_scr", [2, L, 2, UP], F16)

    do = lambda p: p in phases
    with tile.TileContext(nc) as tc, ExitStack() as ctx, \
            nc.allow_low_precision(reason="f32r matmul operands; fp32 psum accum"):
        singles = ctx.enter_context(tc.tile_pool(name="singles", bufs=1))
        pb = ctx.enter_context(tc.tile_pool(name="perbatch", bufs=2))
        xin = ctx.enter_context(tc.tile_pool(name="xin", bufs=4))
        work = ctx.enter_context(tc.tile_pool(name="work", bufs=3))
        etp = ctx.enter_context(tc.tile_pool(name="etp", bufs=6))
        srlp = ctx.enter_context(tc.tile_pool(name="srlp", bufs=12))
        ucp = ctx.enter_context(tc.tile_pool(name="ucp", bufs=6))
        psp = ctx.enter_context(tc.tile_pool(name="psp", bufs=6, space="PSUM"))
        ps_y = ctx.enter_context(tc.tile_pool(name="ps_y", bufs=2, space="PSUM"))
        ps_bld = psp
        ps_att = psp

        # ---- constants / weights ----
        w_sb = singles.tile([128, 8 * 3 * CW], BF16)
        for kb in range(8):
            nc.sync.dma_start(w_sb[:, kb * 384:(kb + 1) * 384],
                              wqkv.ap()[kb * 128:(kb + 1) * 128, :])
        # qkv bias as per-partition columns: bq_cols[p, g] = bqkv[g*128+p]
        # (g: 0=q pre-scaled on host, 1=k, 2=v); applied in the psum->SBUF
        # copies via DVE tensor_scalar, so no bias-init matmuls are needed.
        bq_cols = singles.tile([128, 3], F32)
        nc.sync.dma_start(bq_cols[:], bqkv.ap().rearrange("(g p) -> p g", p=128))
        ertd_sb = singles.tile([128, L], F32R)
        nc.sync.dma_start(ertd_sb[:], ertd.ap())
        wp_sb = singles.tile([CW, D], F32R)
        nc.sync.dma_start(wp_sb[:], wp.ap())
        onesf = singles.tile([128, 512], F32)
        nc.vector.memset(onesf[:], 1.0)
        ones_row = singles.tile([1, 512], F32R)
        nc.vector.tensor_copy(ones_row[:], onesf[0:1, :])
        ident = singles.tile([128, 128], F32)
        make_identity(nc, ident[:])
        ident16 = singles.tile([128, 128], F16)
        make_identity(nc, ident16[:])
        # preset U pad columns to a large negative logit (exp -> 0); finite,
        # not -inf, because the identity inject multiplies pad values by 0.
        padf = singles.tile([128, 2 * 512], F16)
        nc.vector.memset(padf[:], -60000.0)
        for slot in range(2):
            for rg in range(16):
                nc.gpsimd.dma_start(
                    u_scr.ap()[slot, rg * 128:(rg + 1) * 128, :, L:UP], padf[:])

        def emit_build(b):
            """qkv + U emission closures for batch b (allocates its tiles)."""
            st = dict(slot=b % 2)
            st["qT"] = pb.tile([128, L], F32R, tag="qT", name="qT")
            st["kT"] = pb.tile([128, L], F32R, tag="kT", name="kT")
            st["va"] = pb.tile([128, NT * (HS + 1)], F32R, tag="va", name="va")
            st["vb"] = pb.tile([128, NT * (HS + 1)], F32R, tag="vb", name="vb")
            parts = []

            def ones_cols(st=st):
                for vt in (st["va"], st["vb"]):
                    ocol = bass.AP(vt[:].tensor, vt[:].offset + HS,
                                   [vt[:].ap[0], [HS + 1, NT], [1, 1]])
                    nc.vector.tensor_copy(ocol, onesf[:, 0:NT].unsqueeze(2))
            parts.append(ones_cols)

            def qkv_chunk(tch, b=b, st=st):
                qT, kT, va, vb = st["qT"], st["kT"], st["va"], st["vb"]
                col0 = b * L + tch * 512
                xc = xin.tile([128, 8 * 512], BF16, tag="xchunk", name="xc")
                nc.sync.dma_start(
                    xc[:],
                    xT.ap()[:, col0:col0 + 512].rearrange(
                        "(kb p) n -> kb p n", p=128).transpose([1, 0, 2]))
                for m in range(3):      # q, k, v col-groups of 128
                    ps = ps_bld.tile([128, 512], F32, tag="p")
                    for kb in range(8):
                        nc.tensor.matmul(
                            ps[:],
                            w_sb[:, kb * 384 + m * 128: kb * 384 + (m + 1) * 128],
                            xc[:, kb * 512:(kb + 1) * 512],
                            start=(kb == 0), stop=(kb == 7))
                    if m == 0:
                        # qT = ps*SCALE + b_q*SCALE (host pre-scales the q bias)
                        nc.vector.tensor_scalar(
                            qT[:, tch * 512:(tch + 1) * 512], ps[:],
                            SCALE, bq_cols[:, 0:1],
                            mybir.AluOpType.mult, mybir.AluOpType.add)
                    elif m == 1:
                        nc.vector.tensor_scalar_add(
                            kT[:, tch * 512:(tch + 1) * 512], ps[:],
                            bq_cols[:, 1:2])
                    else:
                        vtmp = work.tile([128, 512], F32, tag="vtmp")
                        nc.vector.tensor_scalar_add(vtmp[:], ps[:],
                                                    bq_cols[:, 2:3])
                        for s in range(4):
                            tk = tch * 4 + s
                            pt = ps_bld.tile([128, 512], F32, tag="p")
                            nc.tensor.transpose(pt[:, 0:128],
                                                vtmp[:, s * 128:(s + 1) * 128],
                                                ident[:])
                            nc.vector.tensor_copy(
                                va[:, tk * (HS + 1): tk * (HS + 1) + HS],
                                pt[:, 0:HS])
                            nc.vector.tensor_copy(
                                vb[:, tk * (HS + 1): tk * (HS + 1) + HS],
                                pt[:, HS:2 * HS])

            def u_block(ib, st=st):
                qT, slot = st["qT"], st["slot"]
                i0 = ib * 128
                jbs = list(range(jb_min(ib), 4))
                groups = [jbs[:1], jbs[1:]] if len(jbs) % 2 else                          [jbs[:2], jbs[2:]]
                for grp in groups:
                    if not grp:
                        continue
                    w = 512 * len(grp)
                    ucmb = ucp.tile([128, 2 * 1024], F16, tag="ubf")
                    for gi, jb in enumerate(grp):
                        pua = ps_bld.tile([128, 512], F32, tag="p")
                        pub = ps_bld.tile([128, 512], F32, tag="p")
                        nc.tensor.matmul(pua[:], qT[0:HS, i0:i0 + 128],
                                         ertd_sb[0:HS, jb * 512:(jb + 1) * 512],
                                         start=True, stop=True)
                        nc.tensor.matmul(pub[:], qT[HS:128, i0:i0 + 128],
                                         ertd_sb[HS:128, jb * 512:(jb + 1) * 512],
                                         start=True, stop=True)
                        nc.vector.tensor_copy(
                            ucmb[:, gi * 512: gi * 512 + 512], pua[:])
                        nc.scalar.activation(
                            ucmb[:, w + gi * 512: w + gi * 512 + 512], pub[:],
                            mybir.ActivationFunctionType.Copy)
                    nc.sync.dma_start(
                        u_scr.ap()[slot, i0:i0 + 128, :,
                                   grp[0] * 512: grp[0] * 512 + w],
                        ucmb[:, 0: 2 * w])

            if do("qkv"):
                for tch in range(4):
                    parts.append(lambda tch=tch: qkv_chunk(tch))
                if do("u"):
                    for tch in range(4):
                        parts.append(lambda tch=tch: [u_block(4 * tch + j)
                                                      for j in range(4)])
            return st, parts

        def emit_attn(b, st):
            """attention + projection closures for batch b."""
            qT, kT, va, vb, slot = (st["qT"], st["kT"], st["va"], st["vb"],
                                    st["slot"])
            yn = pb.tile([128, L], F32R, tag="yn")
            parts = []

            def attn_ib(ib5):
                i0 = ib5 * 512
                pyA = ps_y.tile([HS + 1, 512], F32, tag="y")
                pyB = ps_y.tile([HS + 1, 512], F32, tag="y")
                n_mb = 4 * (ib5 + 1)
                for mb in range(n_mb):
                    m0 = mb * 128
                    sss, srels = [], []
                    for h in range(2):
                        ss = ps_att.tile([128, 512], F32, tag="p")
                        nc.tensor.matmul(
                            ss[:], kT[h * HS:(h + 1) * HS, m0:m0 + 128],
                            qT[h * HS:(h + 1) * HS, i0:i0 + 512],
                            start=True, stop=False)
                        base_h = slot * (L * 2 * UP) + h * UP
                        srel = srlp.tile([128, 512], F16, tag="srel")
                        nc.sync.dma_start_transpose(
                            srel[:],
                            bass.AP(u_scr, base_h + (L - 1)
                                    + i0 * (2 * UP - 1) + m0,
                                    [[2 * UP - 1, 512], [1, 128]]))
                        sss.append(ss)
                        srels.append(srel)
                    for ss, srel, py, vt in zip(sss, srels, (pyA, pyB),
                                                (va, vb)):
                        nc.tensor.matmul(ss[:], ident16[:], srel[:],
                                         start=False, stop=True)
                        et = etp.tile([128, 512], F32R, tag="et")
                        nc.scalar.activation(et[:], ss[:],
                                             mybir.ActivationFunctionType.Exp)
                        nc.tensor.matmul(
                            py[:], vt[:, mb * (HS + 1):(mb + 1) * (HS + 1)],
                            et[:], start=(mb == 0), stop=(mb == n_mb - 1))
                for h, py in enumerate((pyA, pyB)):
                    recip = work.tile([1, 512], F32R, tag="recip")
                    nc.vector.reciprocal(recip[:], py[HS:HS + 1, :])
                    pbc = ps_att.tile([128, 512], F32, tag="p")
                    nc.tensor.matmul(pbc[0:HS, :], ones_row[0:1, 0:HS],
                                     recip[:], start=True, stop=True)
                    bc_sb = work.tile([HS, 512], F32, tag="bcsb")
                    nc.scalar.activation(bc_sb[:], pbc[0:HS, :],
                                         mybir.ActivationFunctionType.Copy)
                    nc.vector.tensor_mul(yn[h * HS:(h + 1) * HS, i0:i0 + 512],
                                         py[0:HS, :], bc_sb[:])

            def proj_blk(tkg, b=b):
                for tk in range(4 * tkg, 4 * tkg + 4):
                    t0 = tk * 128
                    osb = work.tile([128, 2 * 512], F16, tag="osb")
                    for nb in range(2):
                        po = ps_att.tile([128, 512], F32, tag="p")
                        nc.tensor.matmul(po[:], yn[:, t0:t0 + 128],
                                         wp_sb[:, nb * 512:(nb + 1) * 512],
                                         start=True, stop=True)
                        nc.vector.tensor_copy(
                            osb[:, nb * 512:(nb + 1) * 512], po[:])
                    nc.sync.dma_start(
                        part.ap()[b * L + t0: b * L + t0 + 128, :], osb[:])

            if do("attn"):
                for ib5 in range(4):
                    parts.append(lambda ib5=ib5: attn_ib(ib5))
                if do("proj"):
                    for tkg in range(4):
                        parts.append(lambda tkg=tkg: proj_blk(tkg))
            elif do("proj"):
                for tkg in range(4):
                    parts.append(lambda tkg=tkg: proj_blk(tkg))
            return parts

        # software-pipelined emission: batch b's attention/projection is
        # interleaved with batch b+1's qkv/U so the scheduler can overlap
        # them across engines.
        def emit_all():
            st, build = emit_build(0)
            for p in build:
                p()
            for b in range(B):
                consume = emit_attn(b, st)
                if b + 1 < B:
                    st, build = emit_build(b + 1)
                else:
                    build = []
                for p in consume + build:
                    p()

        if reps > 1:
            # hardware loop over the whole body: used only by the timing
            # harness (T(reps)-T(1) isolates per-iteration device time from
            # the ~50-100ms axon dispatch overhead)
            with tc.For_i(0, reps):
                emit_all()
        else:
            emit_all()

    return nc


def _round_f32r(a):
    """Round fp32 to fp32r (round-to-nearest-even to 11 mantissa bits) —
    the matmul engine requires f32r operands pre-rounded."""
    b = np.ascontiguousarray(a, np.float32).view(np.uint32)
    r = (b + np.uint32(0x7FF) + ((b >> np.uint32(12)) & np.uint32(1))) \
        & np.uint32(0xFFFFF000)
    return r.view(np.float32)


def make_in_maps(x, W_attn, b_attn, Er, W_proj, b_proj):
    import ml_dtypes
    bf16 = ml_dtypes.bfloat16
    x = np.asarray(x, np.float32)
    W_attn = np.asarray(W_attn, np.float32)
    b_attn = np.asarray(b_attn, np.float32)
    Er = np.asarray(Er, np.float32)
    W_proj = np.asarray(W_proj, np.float32)
    xT = np.ascontiguousarray(x.reshape(TOKS, D).T).astype(bf16)
    ErT = np.ascontiguousarray(Er.T)
    ertd = _round_f32r(np.concatenate([ErT, ErT], axis=0))
    in_maps = []
    for c in range(NCORES):
        q0 = CW * c
        wq = W_attn[:, q0:q0 + CW]
        wk = W_attn[:, D + q0:D + q0 + CW]
        wv = W_attn[:, 2 * D + q0:2 * D + q0 + CW]
        in_maps.append(dict(
            xT=xT,
            wqkv=np.ascontiguousarray(
                np.concatenate([wq, wk, wv], axis=1)).astype(bf16),
            bqkv=np.concatenate(
                [b_attn[q0:q0 + CW] * SCALE, b_attn[D + q0:D + q0 + CW],
                 b_attn[2 * D + q0:2 * D + q0 + CW]]).astype(np.float32),
            ertd=ertd,
            wp=_round_f32r(W_proj[q0:q0 + CW, :]),
        ))
    return in_maps


_cached_nc = None


def kernel(x, W_attn, b_attn, Er, W_proj, b_proj):
    global _cached_nc
    if _cached_nc is None:
        _cached_nc = build_program()
        _split_excess_waits(_cached_nc)
    nc = _cached_nc
    in_maps = make_in_maps(x, W_attn, b_attn, Er, W_proj, b_proj)
    res = bass_utils.run_bass_kernel_spmd(nc, in_maps, list(range(NCORES)))
    out = np.zeros((TOKS, D), np.float32)
    for c in range(NCORES):
        out += res.results[c]["part"].astype(np.float32)
    out += np.asarray(b_proj, np.float32)[None, :]
    return out.reshape(B, L, D)



# revision 22
# speedup vs baseline: 1.2855x; 1.2855x over previous
"""Causal self-attention with relative position (skew trick), 8-way
head-sharded across Trainium2 NeuronCores.  v2.

Shapes (hardcoded): x [4, 2048, 1024], W_attn [1024, 3072], b_attn [3072],
Er [2048, 64], W_proj [1024, 1024], b_proj [1024].  16 heads of 64; each of
the 8 cores handles 2 heads (A, B) for all 4 batches and emits a partial
(pre-reduce) projection output; the host sums the 8 partials and adds b_proj
once.

v2 changes vs v1 (789us): the srel path no longer uses xbar-transposing
DMAs.  U tiles (rows interleaved per head, row pitch 2*UP, pad cols preset
to -60000 = causal mask) are read back with PLAIN strided DMAs in
[query-part, key-free] orientation (full-rate 1KB descriptor runs, both
heads per DMA) and transposed into the S psum by f16 matmuls with the srel
slice stationary and the identity moving -- same PE cost as v1's inject,
~2.5x cheaper on the DMA engines.  PV is reoriented to produce y[query, 65]
psums (65-col f16 matmuls; col 64 = sumexp via the ones-column in V'),
normalized per-partition and PE-transposed back into yn^T for the
projection.  v is computed directly in natural [token, hs] layout by making
the x chunk the stationary matmul operand.  The exp reads a merged 2-head
[128,1024] psum in one ACT op; diagonal 512-blocks skip their causally
masked sub-tiles in S/inject/exp/PV.  DMAs are merged (one U-write per
row-block, one output write per 512 tokens) and PV lags one key-block
behind exp so the PE never waits on ACT.

NOTE: U writes, x loads and output writes go on the sync queue; srel reads
and U-pad writes on gpsimd.  v1 found that issuing u_scr traffic from
nc.scalar corrupts results on hardware even though CoreSim passes.
"""

import numpy as np
from contextlib import ExitStack

import concourse.bass as bass
import concourse.tile as tile
from concourse import mybir
from concourse import bass_utils
from concourse.masks import make_identity
from concourse import library_config

B, L, D = 4, 2048, 1024
NH, HS = 16, 64
NCORES = 8
HPC = 2                 # heads per core
CW = HPC * HS           # 128 head-cols per core
SCALE = 1.0 / 8.0       # 1/sqrt(HS)
F32 = mybir.dt.float32
F32R = mybir.dt.float32r
F16 = mybir.dt.float16
BF16 = mybir.dt.bfloat16
TOKS = B * L
UP = L + 512            # U row pitch; cols [L, UP) = causal-mask pad
NT = L // 128           # token blocks of 128 per batch


# walrus in this toolchain rejects instructions carrying >1 sync-wait;
# move excess waits onto preceding same-engine NOPs.
def _split_excess_waits(nc, max_waits=1):
    for f in nc.m.functions:
        for blk in f.blocks:
            new_insts = []
            for inst in blk.instructions:
                si = getattr(inst, "sync_info", None)
                if si is not None and si.on_wait and len(si.on_wait) > max_waits:
                    waits = list(si.on_wait)
                    chunks = [waits[i:i + max_waits]
                              for i in range(0, len(waits), max_waits)]
                    for j, ch in enumerate(chunks[:-1]):
                        new_insts.append(mybir.InstNoOp(
                            name=f"{inst.name}-waitsplit{j}",
                            engine=inst.engine,
                            sync_info=mybir.SyncInfo(on_wait=ch, on_update=[]),
                            bass_nofuse=True,
                        ))
                    si.on_wait = chunks[-1]
                new_insts.append(inst)
            blk.instructions[:] = new_insts


def jb_min(ib):
    # U row-block ib (128 rows at i0=128*ib) needs Er-index columns
    # j >= 2047 - (i0+127); 512-wide column blocks from jb_min(ib) to 3.
    return max(0, (1920 - 128 * ib) // 512)


def build_program(phases=("qkv", "u", "attn", "proj"), reps=1, vbias=False):
    nc = bass.Bass("TRN2", target_bir_lowering=False, debug=False,
                   num_devices=NCORES)
    xT = nc.declare_dram_parameter("xT", [D, TOKS], BF16, isOutput=False)
    wqkv = nc.declare_dram_parameter("wqkv", [D, 3 * CW], BF16, isOutput=False)
    bqkv = nc.declare_dram_parameter("bqkv", [3 * CW], F32, isOutput=False)
    bvbf = nc.declare_dram_parameter("bvbf", [CW], BF16, isOutput=False)
    ertd = nc.declare_dram_parameter("ertd", [2 * HS, L], F32R, isOutput=False)
    wp = nc.declare_dram_parameter("wp", [CW, D], F32R, isOutput=False)
    part = nc.declare_dram_parameter("part", [TOKS, D], F16, isOutput=True)
    # U scratch: [slot(batch%2), i, head, col]; pitch UP, pad cols hold the
    # causal mask.  Interleaving heads lets one DMA serve both heads' tiles.
    u_scr = nc.dram_tensor("u_scr", [2, L, 2, UP], F16)

    do = lambda p: p in phases
    with tile.TileContext(nc) as tc, ExitStack() as ctx, \
            nc.allow_low_precision(reason="f32r/f16 matmul operands; fp32 psum accum"):
        singles = ctx.enter_context(tc.tile_pool(name="singles", bufs=1))
        pb = ctx.enter_context(tc.tile_pool(name="perbatch", bufs=2))
        xin = ctx.enter_context(tc.tile_pool(name="xin", bufs=3))
        ucp = ctx.enter_context(tc.tile_pool(name="ucp", bufs=2))
        etp = ctx.enter_context(tc.tile_pool(name="etp", bufs=4))
        srp = ctx.enter_context(tc.tile_pool(name="srp", bufs=14))
        wk = ctx.enter_context(tc.tile_pool(name="wk", bufs=2))
        sm = ctx.enter_context(tc.tile_pool(name="sm", bufs=16))
        ps2 = ctx.enter_context(tc.tile_pool(name="ps2", bufs=3, space="PSUM"))
        pspy = ctx.enter_context(tc.tile_pool(name="pspy", bufs=2, space="PSUM"))

        # ---- constants / weights ----
        w_sb = singles.tile([128, 8 * 3 * CW], BF16)
        for kb in range(8):
            nc.sync.dma_start(w_sb[:, kb * 384:(kb + 1) * 384],
                              wqkv.ap()[kb * 128:(kb + 1) * 128, :])
        # qkv bias as per-partition columns: bq_cols[p, g] = bqkv[g*128+p]
        # (g: 0=q pre-scaled on host, 1=k, 2=v); applied in the psum->SBUF
        # copies via DVE tensor_scalar.
        bq_cols = singles.tile([128, 3], F32)
        nc.sync.dma_start(bq_cols[:], bqkv.ap().rearrange("(g p) -> p g", p=128))
        ertd_sb = singles.tile([128, L], F32R)
        nc.sync.dma_start(ertd_sb[:], ertd.ap())
        wp_sb = singles.tile([CW, D], F32R)
        nc.sync.dma_start(wp_sb[:], wp.ap())
        onesf = singles.tile([128, 64], F32)
        nc.vector.memset(onesf[:], 1.0)
        ident16 = singles.tile([128, 128], F16)
        make_identity(nc, ident16[:])
        if vbias:
            bvrow = singles.tile([1, CW], BF16)
            nc.sync.dma_start(bvrow[:], bvbf.ap())
            ones_bf = singles.tile([1, 128], BF16)
            nc.vector.memset(ones_bf[:], 1.0)
        # preset U pad columns to a large negative logit (exp -> 0); finite,
        # not -inf, because the transpose-inject multiplies pads by 0.
        padf = singles.tile([128, 2 * 512], F16)
        nc.vector.memset(padf[:], -60000.0)
        for slot in range(2):
            for rg in range(16):
                nc.gpsimd.dma_start(
                    u_scr.ap()[slot, rg * 128:(rg + 1) * 128, :, L:UP], padf[:])

        # srel tile consumption order for one batch: (ib5, key-512-group,
        # query-sub-block).  Tiles are issued a fixed window ahead of use so
        # the in-order PE stream never waits on a just-issued DMA.
        SR_ORDER = [(ib5, g, isub) for ib5 in range(4)
                    for g in range(ib5 + 1) for isub in range(4)]
        SR_IDX = {key: j for j, key in enumerate(SR_ORDER)}
        SR_W = 8

        def load_srel(slot, ib5, g, isub):
            i0b = (4 * ib5 + isub) * 128
            # diagonal group: key slices beyond isub are causally masked
            # and never read -- trim the transfer.
            cols = (isub + 1) * 128 if g == ib5 else 512
            t = srp.tile([128, 2 * 512], F16, tag="sr")
            base = (slot * (L * 2 * UP) + i0b * (2 * UP)
                    + (L - 1 - i0b) + g * 512)
            src = bass.AP(u_scr, base,
                          [[2 * UP - 1, 128], [UP, 2], [1, cols]])
            # dst [128, 2, cols] is contiguous -> express as 2-D.  The
            # gpsimd SWDGE codegen rejects 3-D DRAM sources ("ISA wrong
            # length"), so srel reads go on the sync (SP/HWDGE) queue,
            # which v1 proved safe for u_scr traffic on hardware.
            nc.sync.dma_start(t[:, 0:2 * cols], src)
            return t, cols

        def sr_ensure(st, upto):
            target = min(len(SR_ORDER), upto + SR_W)
            while st["sr_ptr"] < target:
                key = SR_ORDER[st["sr_ptr"]]
                st["sr_tiles"][key] = load_srel(st["slot"], *key)
                st["sr_ptr"] += 1

        # alternate psum->SBUF evac copies between DVE and ACT
        evac_ctr = [0]

        def evac_copy(dst, src):
            evac_ctr[0] += 1
            if evac_ctr[0] % 2:
                nc.vector.tensor_copy(dst, src)
            else:
                nc.scalar.activation(dst, src,
                                     mybir.ActivationFunctionType.Copy)

        def emit_build(b):
            """qkv + U emission closures for batch b (allocates its tiles)."""
            st = dict(slot=b % 2, sr_tiles={}, sr_ptr=0)
            st["qT"] = pb.tile([128, L], F32R, tag="qT", name="qT")
            st["kT"] = pb.tile([128, L], F32R, tag="kT", name="kT")
            st["va"] = pb.tile([128, NT * (HS + 1)], F16, tag="va", name="va")
            st["vb"] = pb.tile([128, NT * (HS + 1)], F16, tag="vb", name="vb")
            parts = []

            def ones_cols(st=st):
                for vt in (st["va"], st["vb"]):
                    ocol = bass.AP(vt[:].tensor, vt[:].offset + HS,
                                   [vt[:].ap[0], [HS + 1, NT], [1, 1]])
                    nc.vector.tensor_copy(ocol, onesf[:, 0:NT].unsqueeze(2))
            parts.append(ones_cols)

            def qkv_chunk(tch, b=b, st=st):
                qT, kT, va, vb = st["qT"], st["kT"], st["va"], st["vb"]
                col0 = b * L + tch * 512
                xc = xin.tile([128, 8 * 512], BF16, tag="xchunk", name="xc")
                nc.sync.dma_start(
                    xc[:],
                    xT.ap()[:, col0:col0 + 512].rearrange(
                        "(kb p) n -> kb p n", p=128).transpose([1, 0, 2]))
                # q, k: column-major [head-col, token] psums, packed in one
                # 2-bank tile
                pqk = ps2.tile([128, 1024], F32, tag="p2")
                for m in range(2):
                    for kb in range(8):
                        nc.tensor.matmul(
                            pqk[:, m * 512:(m + 1) * 512],
                            w_sb[:, kb * 384 + m * 128: kb * 384 + (m + 1) * 128],
                            xc[:, kb * 512:(kb + 1) * 512],
                            start=(kb == 0), stop=(kb == 7))
                # qT = ps*SCALE + b_q*SCALE (host pre-scales the q bias)
                nc.vector.tensor_scalar(
                    qT[:, tch * 512:(tch + 1) * 512], pqk[:, 0:512],
                    SCALE, bq_cols[:, 0:1],
                    mybir.AluOpType.mult, mybir.AluOpType.add)
                nc.vector.tensor_scalar_add(
                    kT[:, tch * 512:(tch + 1) * 512], pqk[:, 512:1024],
                    bq_cols[:, 1:2])
                # v in natural [token, head-col] layout: x chunk stationary
                # all four s-groups share one psum bank: a start=True zeroes
                # the WHOLE bank, so only the very first matmul starts and
                # only the very last stops (the rest land on pending-zero).
                pv = ps2.tile([128, 1024], F32, tag="p2")
                for s in range(4):
                    for kb in range(8):
                        nc.tensor.matmul(
                            pv[:, s * 128:(s + 1) * 128],
                            xc[:, kb * 512 + s * 128: kb * 512 + (s + 1) * 128],
                            w_sb[:, kb * 384 + 256: kb * 384 + 384],
                            start=(kb == 0 and s == 0),
                            stop=(kb == 7 and s == 3 and not vbias))
                    if vbias:
                        nc.tensor.matmul(pv[:, s * 128:(s + 1) * 128],
                                         ones_bf[0:1, :], bvrow[0:1, :],
                                         start=False, stop=(s == 3))
                for s in range(4):
                    tk = tch * 4 + s
                    nc.vector.tensor_copy(va[:, tk * 65: tk * 65 + 64],
                                          pv[:, s * 128: s * 128 + 64])
                    nc.vector.tensor_copy(vb[:, tk * 65: tk * 65 + 64],
                                          pv[:, s * 128 + 64: s * 128 + 128])

            def u_block(ib, st=st):
                qT, slot = st["qT"], st["slot"]
                i0 = ib * 128
                # exact causal need is cols [L-128*(ib+1), L); chunk from the
                # low end with a partial first chunk, kept >= 256 wide so the
                # f32r matmul stays at 1 cycle/row.
                w_exact = 128 * (ib + 1)
                n512, rem = divmod(w_exact, 512)
                widths = ([512 if rem == 128 else rem] if rem else []) \
                    + [512] * n512
                wt = sum(widths)
                c0 = L - wt
                ucmb = ucp.tile([128, 2 * 2048], F16, tag="ubf")
                off = 0
                for w in widths:
                    c = c0 + off
                    pu = ps2.tile([128, 1024], F32, tag="p2")
                    nc.tensor.matmul(pu[:, 0:w], qT[0:HS, i0:i0 + 128],
                                     ertd_sb[0:HS, c:c + w],
                                     start=True, stop=True)
                    nc.tensor.matmul(pu[:, 512:512 + w], qT[HS:128, i0:i0 + 128],
                                     ertd_sb[HS:128, c:c + w],
                                     start=True, stop=True)
                    # one 3-D copy: psum halves -> ucmb cols {off, wt+off}
                    dst = bass.AP(ucmb[:].tensor, ucmb[:].offset + off,
                                  [ucmb[:].ap[0], [wt, 2], [1, w]])
                    src = bass.AP(pu[:].tensor, pu[:].offset,
                                  [pu[:].ap[0], [512, 2], [1, w]])
                    evac_copy(dst, src)
                    off += w
                dst_d = u_scr.ap()[slot, i0:i0 + 128, :, c0:c0 + wt]
                nc.sync.dma_start(dst_d, ucmb[:, 0:2 * wt])

            if do("qkv"):
                for tch in range(4):
                    parts.append(lambda tch=tch: qkv_chunk(tch))
                if do("u"):
                    for tch in range(4):
                        parts.append(lambda tch=tch: [u_block(4 * tch + j)
                                                      for j in range(4)])
                    if do("attn"):
                        # prefetch the first attention block's srel tiles so
                        # the next batch's consume phase starts with its
                        # inputs already in flight
                        parts.append(lambda st=st: sr_ensure(st, 4))
            return st, parts

        def emit_attn(b, st):
            """attention + projection closures for batch b."""
            qT, kT, va, vb, slot = (st["qT"], st["kT"], st["va"], st["vb"],
                                    st["slot"])
            yn = pb.tile([128, L], F32R, tag="yn")
            pending = []   # deferred normalize/transpose closures

            def flush_norms():
                # pending: [(pyX, iblk0)] per py bank; each bank holds
                # (isl, h) 65-col slices at (isl*2+h)*65, col 64 = sumexp.
                groups = list(pending)
                pending.clear()
                stage2 = []
                for (pyX, iblk0) in groups:
                    rc4 = sm.tile([128, 4], F32, tag="rc")
                    sums = bass.AP(pyX[:].tensor, pyX[:].offset + 64,
                                   [pyX[:].ap[0], [65, 4]])
                    nc.vector.reciprocal(rc4[:], sums)
                    for isl in range(2):
                        ynq2 = sm.tile([128, 128], F16, tag="ynq")
                        for h in range(2):
                            k = isl * 2 + h
                            nc.vector.tensor_scalar_mul(
                                ynq2[:, h * 64:(h + 1) * 64],
                                pyX[:, k * 65:k * 65 + 64], rc4[:, k:k + 1])
                        stage2.append((ynq2, iblk0 + isl))
                for (ynq2, iblk) in stage2:
                    # transpose both heads into one [128,128] psum: rows
                    # 0:64 = head A (partitions 0-63), 64:128 = head B.
                    pyt = pspy.tile([128, 512], F32, tag="py")
                    nc.tensor.matmul(pyt[0:64, 0:128], ynq2[:, 0:64],
                                     ident16[:], start=True, stop=True)
                    nc.tensor.matmul(pyt[64:128, 0:128], ynq2[:, 64:128],
                                     ident16[:], start=True, stop=True)
                    nc.vector.tensor_copy(
                        yn[:, iblk * 128:(iblk + 1) * 128], pyt[:, 0:128])

            def attn_ib(ib5):
                flush_norms()
                i0 = ib5 * 512
                n_mb = 4 * (ib5 + 1)
                pyL = pspy.tile([128, 512], F32, tag="py")
                pyH = pspy.tile([128, 512], F32, tag="py")

                def py_slice(isub, h):
                    pyX = pyL if isub < 2 else pyH
                    return pyX, ((isub % 2) * 2 + h) * 65

                srel_t = st["sr_tiles"]
                sr_ensure(st, SR_IDX[(ib5, 0, 0)] + 4)

                def emit_pv(mbp, et):
                    # pyL holds isub 0,1 x heads; pyH isub 2,3 x heads: one
                    # bank each, so one start (first writer) and one stop
                    # (last writer) per bank; everything else accumulates.
                    kp = mbp - 4 * ib5
                    for h in range(2):
                        vt = va if h == 0 else vb
                        for isub in range(4):
                            if kp > 0 and isub < kp:
                                continue
                            pyX, c = py_slice(isub, h)
                            nc.tensor.matmul(
                                pyX[:, c:c + 65],
                                et[:, h * 512 + isub * 128:
                                   h * 512 + (isub + 1) * 128],
                                vt[:, mbp * 65:(mbp + 1) * 65],
                                start=(mbp == 0 and h == 0 and isub % 2 == 0),
                                stop=(mbp == 4 * ib5 + isub and h == 1
                                      and isub % 2 == 1))

                prev = None
                for mb in range(n_mb):
                    g = mb // 4
                    if mb % 4 == 0:
                        sr_ensure(st, SR_IDX[(ib5, g, 0)] + 4)
                    m0 = mb * 128
                    k = mb - 4 * ib5          # >0 only in the diagonal group
                    cstart = 0 if k <= 0 else min(k, 2) * 128
                    estart = 0 if k <= 0 else k * 128
                    ss = ps2.tile([128, 1024], F32, tag="p2")
                    for h in range(2):
                        nc.tensor.matmul(
                            ss[:, h * 512 + cstart:(h + 1) * 512],
                            kT[h * HS:(h + 1) * HS, m0:m0 + 128],
                            qT[h * HS:(h + 1) * HS, i0 + cstart:i0 + 512],
                            start=True, stop=False)
                    for h in range(2):
                        for isub in range(4):
                            if k > 0 and isub < k:
                                continue
                            t, tcols = srel_t[(ib5, g, isub)]
                            nc.tensor.matmul(
                                ss[:, h * 512 + isub * 128:
                                   h * 512 + (isub + 1) * 128],
                                t[:, h * tcols + (mb % 4) * 128:
                                  h * tcols + (mb % 4 + 1) * 128],
                                ident16[:], start=False, stop=(isub == 3))
                    et = etp.tile([128, 1024], F16, tag="et")
                    if estart:
                        for h in range(2):
                            nc.scalar.activation(
                                et[:, h * 512 + estart:(h + 1) * 512],
                                ss[:, h * 512 + estart:(h + 1) * 512],
                                mybir.ActivationFunctionType.Exp)
                    else:
                        nc.scalar.activation(
                            et[:], ss[:], mybir.ActivationFunctionType.Exp)
                    if prev is not None:
                        emit_pv(*prev)
                    prev = (mb, et)
                emit_pv(*prev)
                pending.append((pyL, 4 * ib5))
                pending.append((pyH, 4 * ib5 + 2))

            def proj_blk(tkg, b=b):
                osb = wk.tile([128, 4 * 1024], F16, tag="osb")
                for j, tk in enumerate(range(4 * tkg, 4 * tkg + 4)):
                    t0 = tk * 128
                    po = ps2.tile([128, 1024], F32, tag="p2")
                    for nb in range(2):
                        nc.tensor.matmul(po[:, nb * 512:(nb + 1) * 512],
                                         yn[:, t0:t0 + 128],
                                         wp_sb[:, nb * 512:(nb + 1) * 512],
                                         start=True, stop=True)
                    evac_copy(osb[:, j * 1024:(j + 1) * 1024], po[:])
                dst = bass.AP(part, (b * L + tkg * 512) * D,
                              [[D, 128], [128 * D, 4], [1, D]])
                src = bass.AP(osb[:].tensor, osb[:].offset,
                              [osb[:].ap[0], [1024, 4], [1, 1024]])
                nc.sync.dma_start(dst, src)

            parts = []
            if do("attn"):
                if do("proj"):
                    parts = [lambda: attn_ib(0), lambda: attn_ib(1),
                             lambda: proj_blk(0), lambda: attn_ib(2),
                             lambda: proj_blk(1), lambda: attn_ib(3),
                             lambda: proj_blk(2), flush_norms,
                             lambda: proj_blk(3)]
                else:
                    parts = [lambda ib5=ib5: attn_ib(ib5) for ib5 in range(4)]
                    parts.append(flush_norms)
            elif do("proj"):
                parts = [lambda tkg=tkg: proj_blk(tkg) for tkg in range(4)]
            return parts

        # software-pipelined emission: batch b's attention/projection parts
        # are round-robin interleaved with batch b+1's qkv/U parts so the
        # scheduler can overlap them across engines.
        def emit_all():
            st, build = emit_build(0)
            for p in build:
                p()
            for b in range(B):
                consume = emit_attn(b, st)
                if b + 1 < B:
                    st, build = emit_build(b + 1)
                else:
                    build = []
                seq = []
                for i in range(max(len(consume), len(build))):
                    if i < len(consume):
                        seq.append(consume[i])
                    if i < len(build):
                        seq.append(build[i])
                for p in seq:
                    p()

        if reps > 1:
            # hardware loop over the whole body: used only by the timing
            # harness (T(reps)-T(1) isolates per-iteration device time from
            # the ~50-100ms axon dispatch overhead)
            with tc.For_i(0, reps):
                emit_all()
        else:
            emit_all()

    return nc


def _round_f32r(a):
    """Round fp32 to fp32r (round-to-nearest-even to 11 mantissa bits) —
    the matmul engine requires f32r operands pre-rounded."""
    b = np.ascontiguousarray(a, np.float32).view(np.uint32)
    r = (b + np.uint32(0x7FF) + ((b >> np.uint32(12)) & np.uint32(1))) \
        & np.uint32(0xFFFFF000)
    return r.view(np.float32)


def make_in_maps(x, W_attn, b_attn, Er, W_proj, b_proj):
    import ml_dtypes
    bf16 = ml_dtypes.bfloat16
    x = np.asarray(x, np.float32)
    W_attn = np.asarray(W_attn, np.float32)
    b_attn = np.asarray(b_attn, np.float32)
    Er = np.asarray(Er, np.float32)
    W_proj = np.asarray(W_proj, np.float32)
    xT = np.ascontiguousarray(x.reshape(TOKS, D).T).astype(bf16)
    ErT = np.ascontiguousarray(Er.T)
    ertd = _round_f32r(np.concatenate([ErT, ErT], axis=0))
    in_maps = []
    for c in range(NCORES):
        q0 = CW * c
        wq = W_attn[:, q0:q0 + CW]
        wk = W_attn[:, D + q0:D + q0 + CW]
        wv = W_attn[:, 2 * D + q0:2 * D + q0 + CW]
        in_maps.append(dict(
            xT=xT,
            wqkv=np.ascontiguousarray(
                np.concatenate([wq, wk, wv], axis=1)).astype(bf16),
            bqkv=np.concatenate(
                [b_attn[q0:q0 + CW] * SCALE, b_attn[D + q0:D + q0 + CW],
                 b_attn[2 * D + q0:2 * D + q0 + CW]]).astype(np.float32),
            bvbf=b_attn[2 * D + q0:2 * D + q0 + CW].astype(bf16),
            ertd=ertd,
            wp=_round_f32r(W_proj[q0:q0 + CW, :]),
        ))
    return in_maps


_cached_nc = {}


def kernel(x, W_attn, b_attn, Er, W_proj, b_proj):
    vbias = bool(np.any(np.asarray(b_attn)[2 * D:]))
    if vbias not in _cached_nc:
        nc = build_program(vbias=vbias)
        _split_excess_waits(nc)
        _cached_nc[vbias] = nc
    nc = _cached_nc[vbias]
    in_maps = make_in_maps(x, W_attn, b_attn, Er, W_proj, b_proj)
    res = bass_utils.run_bass_kernel_spmd(nc, in_maps, list(range(NCORES)))
    out = np.zeros((TOKS, D), np.float32)
    for c in range(NCORES):
        out += res.results[c]["part"].astype(np.float32)
    out += np.asarray(b_proj, np.float32)[None, :]
    return out.reshape(B, L, D)
